# revision 1
# baseline (speedup 1.0000x reference)
"""Conv2d 3x3 (stride 1, pad 1) forward on 8 Trainium2 NeuronCores.

Problem: x (32,32,128,128) f32, kernel (64,32,3,3), bias (64)
         -> out (32,64,128,128).  Data-parallel: 4 images per core.

Per-core design:
  - Each of the 4 images is pinned to one 32-partition PE row group
    (K = Cin = 32).  Its zero-padded activations live at partitions
    32r..32r+32 as [Cin, H+2, W+2] fp32r.
  - A 3x3 conv is 9 shifted matmuls accumulated in PSUM: for tap
    (kh,kw), out[co, h, w] += W_t[ci,co] * xpad[ci, h+kh, w+kw].
    Per round we produce 4 output rows (N = 4*128 = 512 = 1 PSUM bank)
    for every image; the 4 matmul streams run on disjoint 32x64 PE
    tiles (tile_position (32r, 0)) concurrently.
  - fp32r runs the PE at 1 cycle/column for N>=256 (vs 4 for fp32).
  - Drain: ScalarE handles images 0,1 and VectorE images 2,3, adding
    bias while copying PSUM -> SBUF.  Images 1,3 are written with a
    +64 partition shift so each staging tile spans all 128 partitions,
    keeping the HBM store DMAs at full port bandwidth.
"""
import sys
sys.path.insert(0, '/opt/trn_rl_repo')
import numpy as np

B, Cin, H, W = 32, 32, 128, 128
Cout, KH, KW = 64, 3, 3
NCORES = 8
BPC = B // NCORES          # images per core
Hp, Wp = H + 2, W + 2
NTAP = KH * KW
ROWS_PER_ROUND = 4
NROUND = H // ROWS_PER_ROUND

_cache = {}


def _build_program():
    from concourse import bacc
    import concourse.mybir as mybir
    from concourse.tile import TileContext

    f32 = mybir.dt.float32
    f32r = mybir.dt.float32r
    Act = mybir.ActivationFunctionType

    nc = bacc.Bacc("TRN2", target_bir_lowering=False, debug=False,
                   num_devices=NCORES)
    x_ext = nc.declare_dram_parameter("x", [BPC * Cin, H, W], f32r,
                                      isOutput=False)
    w_ext = nc.declare_dram_parameter("w", [128, NTAP, Cout], f32r,
                                      isOutput=False)
    b_ext = nc.declare_dram_parameter("b", [128, 1], f32, isOutput=False)
    out_ext = nc.declare_dram_parameter("out", [BPC * Cout, H, W], f32,
                                        isOutput=True)

    with TileContext(nc) as tc:
        with tc.tile_pool(name="xp", bufs=1) as xpool, \
             tc.tile_pool(name="const", bufs=1) as cpool, \
             tc.tile_pool(name="stage", bufs=6) as opool, \
             tc.tile_pool(name="psum", bufs=8, space="PSUM") as ppool:

            xp = xpool.tile([128, Hp, Wp], f32r)
            wt = cpool.tile([128, NTAP, Cout], f32r)
            bt = cpool.tile([128, 1], f32)

            nc.sync.dma_start(out=wt[:], in_=w_ext[:])
            nc.sync.dma_start(out=bt[:], in_=b_ext[:])

            # zero the one-pixel halo (rows 0 / Hp-1, cols 0 / Wp-1)
            nc.vector.memset(xp[:, 0, :].bitcast(f32), 0.0)
            nc.vector.memset(xp[:, Hp - 1, :].bitcast(f32), 0.0)
            nc.vector.memset(xp[:, :, 0].bitcast(f32), 0.0)
            nc.vector.memset(xp[:, :, Wp - 1].bitcast(f32), 0.0)

            # interior, chunked so early rounds can start before the
            # whole image landed
            XCH = 4
            rows_per_ch = H // XCH
            for g in range(XCH):
                r0 = g * rows_per_ch
                nc.sync.dma_start(
                    out=xp[:, 1 + r0:1 + r0 + rows_per_ch, 1:1 + W],
                    in_=x_ext[:, r0:r0 + rows_per_ch, :])

            out_v = out_ext.rearrange(
                "(pair half co) h w -> (half co) pair (h w)",
                pair=2, half=2, co=Cout)

            for k in range(NROUND):
                h0 = k * ROWS_PER_ROUND
                ps = [ppool.tile([Cout, ROWS_PER_ROUND, W], f32, tag="ps",
                                 name=f"ps{k}_{r}")
                      for r in range(BPC)]
                for t in range(NTAP):
                    kh, kw = divmod(t, 3)
                    for r in range(BPC):
                        nc.tensor.matmul(
                            ps[r][:, :, :],
                            wt[32 * r:32 * r + 32, t, :],
                            xp[32 * r:32 * r + 32,
                               h0 + kh:h0 + kh + ROWS_PER_ROUND,
                               kw:kw + W],
                            start=(t == 0), stop=(t == NTAP - 1),
                            tile_position=(32 * r, 0))

                ost = opool.tile([128, 2, ROWS_PER_ROUND, W], f32, tag="ost")
                # ScalarE: images 0,1 (image 1 shifted to partitions 64..128)
                nc.scalar.activation(ost[0:64, 0, :, :], ps[0][:, :, :],
                                     Act.Identity, bias=bt[0:64, :])
                nc.scalar.activation(ost[64:128, 0, :, :], ps[1][:, :, :],
                                     Act.Identity, bias=bt[64:128, :])
                # VectorE: images 2,3
                nc.vector.tensor_scalar_add(ost[0:64, 1, :, :], ps[2][:, :, :],
                                            bt[0:64, :])
                nc.vector.tensor_scalar_add(ost[64:128, 1, :, :], ps[3][:, :, :],
                                            bt[64:128, :])
                # one 1-MiB store for all 4 images' 4 rows
                nc.sync.dma_start(
                    out=out_v[:, :, h0 * W:(h0 + ROWS_PER_ROUND) * W],
                    in_=ost[:, :, :, :])

    nc.compile()
    return nc


def _get_program():
    if "nc" not in _cache:
        _cache["nc"] = _build_program()
    return _cache["nc"]


def _prep_inputs(x, kernel, bias):
    # weights: (Cout, Cin, KH, KW) -> [ci, tap, co], replicated on the
    # 4 PE row groups
    w = np.transpose(kernel.reshape(Cout, Cin, NTAP), (1, 2, 0))
    w = np.ascontiguousarray(np.tile(w, (4, 1, 1)), dtype=np.float32)
    b = np.ascontiguousarray(
        np.tile(bias.astype(np.float32), 2)[:, None])
    in_maps = []
    for c in range(NCORES):
        xs = np.ascontiguousarray(
            x[c * BPC:(c + 1) * BPC].reshape(BPC * Cin, H, W),
            dtype=np.float32)
        in_maps.append({"x": xs, "w": w, "b": b})
    return in_maps


def _run(inputs, trace=False):
    from concourse.bass_utils import run_bass_kernel_spmd
    nc = _get_program()
    in_maps = _prep_inputs(inputs["x"], inputs["kernel"], inputs["bias"])
    res = run_bass_kernel_spmd(nc, in_maps, list(range(NCORES)), trace=trace)
    out = np.concatenate(
        [res.results[c]["out"].reshape(BPC, Cout, H, W)
         for c in range(NCORES)], axis=0)
    return out.astype(np.float32), res


def kernel(**inputs):
    out, _ = _run(inputs, trace=False)
    return out



# revision 2
# speedup vs baseline: 1.2307x; 1.2307x over previous
"""Conv2d 3x3 (stride 1, pad 1) forward on 8 Trainium2 NeuronCores.

Problem: x (32,32,128,128) f32, kernel (64,32,3,3), bias (64)
         -> out (32,64,128,128).  Data-parallel: 4 images per core.

v2 design (target: memory roofline):
  - All off-chip traffic is fp16 (tolerance 2e-2; fp16 conv err ~3e-4):
    x is cast+zero-padded on the host to [128, 130, 130] f16 per core so
    the load is 4.1 MiB of fully contiguous descriptors (no memsets, no
    strided 512B writes).  Output is stored as f16 in a device-friendly
    linear layout (8 MiB instead of 16) and un-permuted on the host.
  - Compute: per round, 4 output rows per image; 9 shifted matmuls
    accumulate in PSUM.  Images 0,1 on PE column groups 0-1
    (tile_position (32r, 0)), images 2,3 on column groups 2-3
    (tile_position (32r, 64)) so that one PSUM bank holds TWO images
    (partitions 0-63 / 64-127).  The 4 matmul streams still run
    concurrently on the 4 row groups.
  - Drain: per round just TWO full-width 128-partition ops:
    ScalarE handles bank pair0 (imgs 0,2), VectorE bank pair1 (imgs 1,3),
    adding bias and casting f32->f16 on the way out.
  - Stores: 4 rounds staged per DMA -> 8 stores of 1 MiB with 8 KiB
    per-partition descriptors.
"""
import sys
sys.path.insert(0, '/opt/trn_rl_repo')
import numpy as np

B, Cin, H, W = 32, 32, 128, 128
Cout, KH, KW = 64, 3, 3
NCORES = 8
BPC = B // NCORES          # images per core
Hp, Wp = H + 2, W + 2
NTAP = KH * KW
ROWS = 4                   # output rows per round
NROUND = H // ROWS
RB = 4                     # rounds staged per store DMA

_cache = {}


def _build_program():
    from concourse import bacc
    import concourse.mybir as mybir
    from concourse.tile import TileContext

    f32 = mybir.dt.float32
    f16 = mybir.dt.float16
    Act = mybir.ActivationFunctionType

    nc = bacc.Bacc("TRN2", target_bir_lowering=False, debug=False,
                   num_devices=NCORES)
    x_ext = nc.declare_dram_parameter("x", [128, Hp, Wp], f16, isOutput=False)
    w_ext = nc.declare_dram_parameter("w", [128, NTAP, Cout], f16,
                                      isOutput=False)
    b_ext = nc.declare_dram_parameter("b", [128, 1], f32, isOutput=False)
    # out[p, k, pair, rho, w]: p = 64*ph + co; image = 2*ph + pair;
    # h = ROWS*k + rho
    out_ext = nc.declare_dram_parameter("out", [128, NROUND, 2, ROWS, W], f16,
                                        isOutput=True)

    with TileContext(nc) as tc:
        with tc.tile_pool(name="xp", bufs=1) as xpool, \
             tc.tile_pool(name="const", bufs=1) as cpool, \
             tc.tile_pool(name="stage", bufs=3) as opool, \
             tc.tile_pool(name="psum", bufs=8, space="PSUM") as ppool:

            xp = xpool.tile([128, Hp, Wp], f16)
            wt = cpool.tile([128, NTAP, Cout], f16)
            bt = cpool.tile([128, 1], f32)

            nc.sync.dma_start(out=wt[:], in_=w_ext[:])
            nc.sync.dma_start(out=bt[:], in_=b_ext[:])

            # x load in 4 row-chunks so early rounds start promptly
            bounds = [0, 33, 65, 97, Hp]
            for g in range(4):
                a, b = bounds[g], bounds[g + 1]
                nc.sync.dma_start(out=xp[:, a:b, :], in_=x_ext[:, a:b, :])

            for gk in range(NROUND // RB):
                ost = opool.tile([128, RB, 2, ROWS, W], f16, tag="ost")
                for k in range(gk * RB, (gk + 1) * RB):
                    h0 = k * ROWS
                    slot = k % RB
                    ps0 = ppool.tile([128, ROWS, W], f32, tag="ps",
                                     name=f"ps{k}_0")   # imgs 0,2
                    ps1 = ppool.tile([128, ROWS, W], f32, tag="ps",
                                     name=f"ps{k}_1")   # imgs 1,3
                    for t in range(NTAP):
                        kh, kw = divmod(t, 3)
                        st, sp = (t == 0), (t == NTAP - 1)
                        rows = xp[:, h0 + kh:h0 + kh + ROWS, kw:kw + W]
                        nc.tensor.matmul(ps0[0:64], wt[0:32, t, :],
                                         rows[0:32], start=st, stop=sp,
                                         tile_position=(0, 0))
                        nc.tensor.matmul(ps1[0:64], wt[32:64, t, :],
                                         rows[32:64], start=st, stop=sp,
                                         tile_position=(32, 0))
                        nc.tensor.matmul(ps0[64:128], wt[64:96, t, :],
                                         rows[64:96], start=st, stop=sp,
                                         tile_position=(64, 64))
                        nc.tensor.matmul(ps1[64:128], wt[96:128, t, :],
                                         rows[96:128], start=st, stop=sp,
                                         tile_position=(96, 64))
                    # full-width drains: +bias, cast f32->f16
                    nc.scalar.activation(ost[:, slot, 0, :, :], ps0[:, :, :],
                                         Act.Identity, bias=bt[:, :])
                    nc.vector.tensor_scalar_add(ost[:, slot, 1, :, :],
                                                ps1[:, :, :], bt[:, :])
                nc.sync.dma_start(
                    out=out_ext[:, gk * RB:(gk + 1) * RB, :, :, :],
                    in_=ost[:, :, :, :, :])

    nc.compile()
    return nc


def _get_program():
    if "nc" not in _cache:
        _cache["nc"] = _build_program()
    return _cache["nc"]


def _prep_inputs(x, kernel, bias):
    # weights: (Cout, Cin, KH, KW) -> [ci, tap, co], replicated on the
    # 4 PE row groups
    w = np.transpose(kernel.reshape(Cout, Cin, NTAP), (1, 2, 0))
    w = np.ascontiguousarray(np.tile(w, (4, 1, 1))).astype(np.float16)
    b = np.ascontiguousarray(
        np.tile(bias.astype(np.float32), 2)[:, None])
    x16 = x.astype(np.float16)
    in_maps = []
    for c in range(NCORES):
        xs = np.zeros((128, Hp, Wp), dtype=np.float16)
        xs[:, 1:1 + H, 1:1 + W] = x16[c * BPC:(c + 1) * BPC].reshape(
            BPC * Cin, H, W)
        in_maps.append({"x": xs, "w": w, "b": b})
    return in_maps


def _unshard(res):
    outs = []
    for c in range(NCORES):
        a = res.results[c]["out"]          # [128, NROUND, 2, ROWS, W] f16
        a = a.reshape(2, Cout, NROUND, 2, ROWS, W)   # [ph, co, k, pair, rho, w]
        a = np.transpose(a, (0, 3, 1, 2, 4, 5))      # [ph, pair, co, k, rho, w]
        outs.append(a.reshape(BPC, Cout, H, W))
    return np.concatenate(outs, axis=0).astype(np.float32)


def _run(inputs, trace=False):
    from concourse.bass_utils import run_bass_kernel_spmd
    nc = _get_program()
    in_maps = _prep_inputs(inputs["x"], inputs["kernel"], inputs["bias"])
    res = run_bass_kernel_spmd(nc, in_maps, list(range(NCORES)), trace=trace)
    return _unshard(res), res


def kernel(**inputs):
    out, _ = _run(inputs, trace=False)
    return out


# revision 4
# speedup vs baseline: 1.2469x; 1.0131x over previous
"""Conv2d 3x3 (stride 1, pad 1) forward on 8 Trainium2 NeuronCores.

Problem: x (32,32,128,128) f32, kernel (64,32,3,3), bias (64)
         -> out (32,64,128,128).  Data-parallel: 4 images per core.

v5 design (round-pair, 8 concurrent PE streams):
  - All off-chip traffic is fp16 (tolerance 2e-2; fp16 conv err ~3e-4).
    x is cast + zero-padded on the host to [128, 130, 130] f16 per core;
    output is f16 in a device-friendly layout, un-permuted on the host.
  - Each 32-partition row group r (image r) feeds BOTH 64-wide column
    halves of the PE concurrently: tile (32r, 0) computes round kA while
    tile (32r, 64) computes round kB = kA+1 with the SAME weights.
    8 concurrent matmul streams -> 2x the column rate of the 4-stream
    variant.  One PSUM bank holds both rounds of one image (partitions
    0-63 / 64-127), so drains stay full-width 128-partition ops.
  - Blocks of 4 rounds (2 round-pairs) x 8 banks use all of PSUM; each
    tap's weights are loaded once per tile per block (2 matmuls per
    LDWEIGHTS when walrus dedupes adjacent same-weight matmuls).
  - ScalarE drains images 0,1; VectorE images 2,3 (+bias, f32->f16).
  - Head: tiny first x chunk + PE warm-up matmuls on zeros so the HAM
    clock gate is released before real matmuls start.
  - Tail: final block stored in 4 small DMAs to shorten the last
    store+completion chain.
"""
import sys
sys.path.insert(0, '/opt/trn_rl_repo')
import numpy as np

B, Cin, H, W = 32, 32, 128, 128
Cout, KH, KW = 64, 3, 3
NCORES = 8
BPC = B // NCORES          # images per core
Hp, Wp = H + 2, W + 2
NTAP = KH * KW
ROWS = 4                   # output rows per round
NROUND = H // ROWS
NBLK = NROUND // 4         # blocks of 4 rounds (= 2 round-pairs)
NWARM = 10                 # PE warm-up matmuls

_cache = {}


def _build_program():
    from concourse import bacc
    import concourse.mybir as mybir
    from concourse.tile import TileContext

    f32 = mybir.dt.float32
    f16 = mybir.dt.float16
    Act = mybir.ActivationFunctionType

    nc = bacc.Bacc("TRN2", target_bir_lowering=False, debug=False,
                   num_devices=NCORES)
    x_ext = nc.declare_dram_parameter("x", [128, Hp, Wp], f16, isOutput=False)
    w_ext = nc.declare_dram_parameter("w", [128, NTAP, Cout], f16,
                                      isOutput=False)
    b_ext = nc.declare_dram_parameter("b", [128, 1], f32, isOutput=False)
    # out[p, g, r, j, rho, w]: p = 64*rh + co; img = r;
    # h = 4*(4g + 2j + rh) + rho
    out_ext = nc.declare_dram_parameter(
        "out", [128, NBLK, BPC, 2, ROWS, W], f16, isOutput=True)

    with TileContext(nc) as tc:
        with tc.tile_pool(name="xp", bufs=1) as xpool, \
             tc.tile_pool(name="const", bufs=1) as cpool, \
             tc.tile_pool(name="stage", bufs=3) as opool, \
             tc.tile_pool(name="psum", bufs=8, space="PSUM") as ppool:

            xp = xpool.tile([128, Hp, Wp], f16)
            wt = cpool.tile([128, NTAP, Cout], f16)
            bt = cpool.tile([128, 1], f32)
            zt = cpool.tile([32, 512], f16)

            nc.sync.dma_start(out=wt[:], in_=w_ext[:])
            nc.sync.dma_start(out=bt[:], in_=b_ext[:])

            # x load: tiny first chunk so block 0 starts promptly
            bounds = [0, 19, 51, 83, 115, Hp]
            for g in range(len(bounds) - 1):
                a, b = bounds[g], bounds[g + 1]
                nc.sync.dma_start(out=xp[:, a:b, :], in_=x_ext[:, a:b, :])

            # PE warm-up: release the HAM clock gate during the x load.
            nc.vector.memset(zt[:].bitcast(f32), 0.0)
            pw = ppool.tile([128, ROWS, W], f32, tag="ps", name="warm")
            for i in range(NWARM):
                nc.tensor.matmul(pw[0:64], zt[0:32, 0:64], zt[0:32, :],
                                 start=True, stop=True, tile_position=(0, 0),
                                 skip_group_check=True)

            for g in range(NBLK):
                ost = opool.tile([128, BPC, 2, ROWS, W], f16, tag="ost")
                ps = [[ppool.tile([128, ROWS, W], f32, tag="ps",
                                  name=f"ps{g}_{r}_{j}")
                       for j in range(2)] for r in range(BPC)]
                for t in range(NTAP):
                    kh, kw = divmod(t, 3)
                    st, sp = (t == 0), (t == NTAP - 1)
                    for r in range(BPC):
                        w_sl = wt[32 * r:32 * r + 32, t, :]
                        for cg in (0, 64):   # column half; rh = round parity
                            rh = 0 if cg == 0 else 1
                            for j in range(2):
                                k = 4 * g + 2 * j + rh
                                h0 = 4 * k
                                nc.tensor.matmul(
                                    ps[r][j][cg:cg + 64],
                                    w_sl,
                                    xp[32 * r:32 * r + 32,
                                       h0 + kh:h0 + kh + ROWS, kw:kw + W],
                                    start=st, stop=sp,
                                    tile_position=(32 * r, cg))
                # full-width drains: +bias, cast f32->f16
                for r in range(BPC):
                    eng = nc.scalar if r < 2 else nc.vector
                    for j in range(2):
                        if r < 2:
                            eng.activation(ost[:, r, j, :, :], ps[r][j][:],
                                           Act.Identity, bias=bt[:, :])
                        else:
                            eng.tensor_scalar_add(ost[:, r, j, :, :],
                                                  ps[r][j][:], bt[:, :])
                if g < NBLK - 1:
                    nc.sync.dma_start(out=out_ext[:, g], in_=ost[:])
                else:
                    # split the final store to shorten the tail
                    for r in range(BPC):
                        nc.sync.dma_start(out=out_ext[:, g, r],
                                          in_=ost[:, r])

    nc.compile()
    return nc


def _get_program():
    if "nc" not in _cache:
        _cache["nc"] = _build_program()
    return _cache["nc"]


def _prep_inputs(x, kernel, bias):
    # weights: (Cout, Cin, KH, KW) -> [ci, tap, co], replicated on the
    # 4 PE row groups
    w = np.transpose(kernel.reshape(Cout, Cin, NTAP), (1, 2, 0))
    w = np.ascontiguousarray(np.tile(w, (4, 1, 1))).astype(np.float16)
    b = np.ascontiguousarray(
        np.tile(bias.astype(np.float32), 2)[:, None])
    x16 = x.astype(np.float16)
    in_maps = []
    for c in range(NCORES):
        xs = np.zeros((128, Hp, Wp), dtype=np.float16)
        xs[:, 1:1 + H, 1:1 + W] = x16[c * BPC:(c + 1) * BPC].reshape(
            BPC * Cin, H, W)
        in_maps.append({"x": xs, "w": w, "b": b})
    return in_maps


def _unshard(res):
    outs = []
    for c in range(NCORES):
        a = res.results[c]["out"]   # [128, NBLK, BPC, 2, ROWS, W] f16
        a = a.reshape(2, Cout, NBLK, BPC, 2, ROWS, W)  # [rh,co,g,r,j,rho,w]
        a = np.transpose(a, (3, 1, 2, 4, 0, 5, 6))     # [r,co,g,j,rh,rho,w]
        outs.append(a.reshape(BPC, Cout, H, W))
    return np.concatenate(outs, axis=0).astype(np.float32)


def _run(inputs, trace=False):
    from concourse.bass_utils import run_bass_kernel_spmd
    nc = _get_program()
    in_maps = _prep_inputs(inputs["x"], inputs["kernel"], inputs["bias"])
    res = run_bass_kernel_spmd(nc, in_maps, list(range(NCORES)), trace=trace)
    return _unshard(res), res


def kernel(**inputs):
    out, _ = _run(inputs, trace=False)
    return out


# revision 5
# speedup vs baseline: 1.2792x; 1.0259x over previous
"""Conv2d 3x3 (stride 1, pad 1) forward on 8 Trainium2 NeuronCores.

Problem: x (32,32,128,128) f32, kernel (64,32,3,3), bias (64)
         -> out (32,64,128,128).  Data-parallel: 4 images per core.

v6 design (at the PE feed-rate roofline):
  - All off-chip traffic is fp16 (tolerance 2e-2; fp16 conv err ~3e-4).
    x is cast + zero-padded on the host to [128, 130, 130] f16 per core
    so the load is 4.1 MiB of contiguous 8KB descriptors; the output is
    f16 in a device-friendly layout (8 MiB), un-permuted on the host.
  - Compute: per round 4 output rows/image; 9 shifted matmuls accumulate
    in PSUM; 4 concurrent streams on the 4 PE row groups (the SBUF feed
    rate of 1 elem/partition/cycle makes this the max useful rate for
    Cout=64).  Images 0,1 output on column groups 0-1, images 2,3 on
    column groups 2-3, so one PSUM bank holds two images and the drains
    are full-width: per round one ScalarE activation (+bias, ->f16) and
    one VectorE tensor_scalar_add.
  - Head: tiny first x chunk [0,19) + 7 warm-up matmuls on zeros so the
    HAM clock gate is released and the PE is hot when real data lands.
  - Tail: stores grouped 4+4+...+2+1+1 rounds so the final store chain
    after the last matmul is short.
"""
import sys
sys.path.insert(0, '/opt/trn_rl_repo')
import numpy as np

B, Cin, H, W = 32, 32, 128, 128
Cout, KH, KW = 64, 3, 3
NCORES = 8
BPC = B // NCORES          # images per core
Hp, Wp = H + 2, W + 2
NTAP = KH * KW
ROWS = 4                   # output rows per round
NROUND = H // ROWS
NWARM = 7                  # PE warm-up matmuls
GROUPS = [(0, 4), (4, 8), (8, 12), (12, 16), (16, 20), (20, 24),
          (24, 28), (28, 30), (30, 31), (31, 32)]

_cache = {}


def _build_program():
    from concourse import bacc
    import concourse.mybir as mybir
    from concourse.tile import TileContext

    f32 = mybir.dt.float32
    f16 = mybir.dt.float16
    Act = mybir.ActivationFunctionType

    nc = bacc.Bacc("TRN2", target_bir_lowering=False, debug=False,
                   num_devices=NCORES)
    x_ext = nc.declare_dram_parameter("x", [128, Hp, Wp], f16, isOutput=False)
    w_ext = nc.declare_dram_parameter("w", [128, NTAP, Cout], f16,
                                      isOutput=False)
    b_ext = nc.declare_dram_parameter("b", [128, 1], f32, isOutput=False)
    # out[p, k, pair, rho, w]: p = 64*ph + co; image = 2*ph + pair;
    # h = ROWS*k + rho
    out_ext = nc.declare_dram_parameter(
        "out", [128, NROUND, 2, ROWS, W], f16, isOutput=True)

    with TileContext(nc) as tc:
        with tc.tile_pool(name="xp", bufs=1) as xpool, \
             tc.tile_pool(name="const", bufs=1) as cpool, \
             tc.tile_pool(name="stage", bufs=3) as opool, \
             tc.tile_pool(name="psum", bufs=8, space="PSUM") as ppool:

            xp = xpool.tile([128, Hp, Wp], f16)
            wt = cpool.tile([128, NTAP, Cout], f16)
            bt = cpool.tile([128, 1], f32)
            zt = cpool.tile([32, 512], f16)

            nc.sync.dma_start(out=wt[:], in_=w_ext[:])
            nc.sync.dma_start(out=bt[:], in_=b_ext[:])

            # x load: tiny first chunk so round 0 starts promptly
            bounds = [0, 19, 51, 83, 115, Hp]
            for g in range(len(bounds) - 1):
                a, b = bounds[g], bounds[g + 1]
                nc.sync.dma_start(out=xp[:, a:b, :], in_=x_ext[:, a:b, :])

            # PE warm-up: release the HAM clock gate during the x load.
            nc.vector.memset(zt[:].bitcast(f32), 0.0)
            pw = ppool.tile([128, ROWS, W], f32, tag="ps", name="warm")
            for i in range(NWARM):
                nc.tensor.matmul(pw[0:64], zt[0:32, 0:64], zt[0:32, :],
                                 start=True, stop=True, tile_position=(0, 0),
                                 skip_group_check=True)

            for (k0, k1) in GROUPS:
                ost = opool.tile([128, k1 - k0, 2, ROWS, W], f16, tag="ost")
                for k in range(k0, k1):
                    h0 = k * ROWS
                    slot = k - k0
                    ps0 = ppool.tile([128, ROWS, W], f32, tag="ps",
                                     name=f"ps{k}_0")   # imgs 0,2
                    ps1 = ppool.tile([128, ROWS, W], f32, tag="ps",
                                     name=f"ps{k}_1")   # imgs 1,3
                    for t in range(NTAP):
                        kh, kw = divmod(t, 3)
                        st, sp = (t == 0), (t == NTAP - 1)
                        rows = xp[:, h0 + kh:h0 + kh + ROWS, kw:kw + W]
                        nc.tensor.matmul(ps0[0:64], wt[0:32, t, :],
                                         rows[0:32], start=st, stop=sp,
                                         tile_position=(0, 0))
                        nc.tensor.matmul(ps1[0:64], wt[32:64, t, :],
                                         rows[32:64], start=st, stop=sp,
                                         tile_position=(32, 0))
                        nc.tensor.matmul(ps0[64:128], wt[64:96, t, :],
                                         rows[64:96], start=st, stop=sp,
                                         tile_position=(64, 64))
                        nc.tensor.matmul(ps1[64:128], wt[96:128, t, :],
                                         rows[96:128], start=st, stop=sp,
                                         tile_position=(96, 64))
                    # full-width drains: +bias, cast f32->f16
                    nc.scalar.activation(ost[:, slot, 0, :, :], ps0[:, :, :],
                                         Act.Identity, bias=bt[:, :])
                    nc.vector.tensor_scalar_add(ost[:, slot, 1, :, :],
                                                ps1[:, :, :], bt[:, :])
                nc.sync.dma_start(out=out_ext[:, k0:k1], in_=ost[:])

    nc.compile()
    return nc


def _get_program():
    if "nc" not in _cache:
        _cache["nc"] = _build_program()
    return _cache["nc"]


def _prep_inputs(x, kernel, bias):
    # weights: (Cout, Cin, KH, KW) -> [ci, tap, co], replicated on the
    # 4 PE row groups
    w = np.transpose(kernel.reshape(Cout, Cin, NTAP), (1, 2, 0))
    w = np.ascontiguousarray(np.tile(w, (4, 1, 1))).astype(np.float16)
    b = np.ascontiguousarray(
        np.tile(bias.astype(np.float32), 2)[:, None])
    x16 = x.astype(np.float16)
    in_maps = []
    for c in range(NCORES):
        xs = np.zeros((128, Hp, Wp), dtype=np.float16)
        xs[:, 1:1 + H, 1:1 + W] = x16[c * BPC:(c + 1) * BPC].reshape(
            BPC * Cin, H, W)
        in_maps.append({"x": xs, "w": w, "b": b})
    return in_maps


def _unshard(res):
    outs = []
    for c in range(NCORES):
        a = res.results[c]["out"]          # [128, NROUND, 2, ROWS, W] f16
        a = a.reshape(2, Cout, NROUND, 2, ROWS, W)   # [ph, co, k, pair, rho, w]
        a = np.transpose(a, (0, 3, 1, 2, 4, 5))      # [ph, pair, co, k, rho, w]
        outs.append(a.reshape(BPC, Cout, H, W))
    return np.concatenate(outs, axis=0).astype(np.float32)


def _run(inputs, trace=False):
    from concourse.bass_utils import run_bass_kernel_spmd
    nc = _get_program()
    in_maps = _prep_inputs(inputs["x"], inputs["kernel"], inputs["bias"])
    res = run_bass_kernel_spmd(nc, in_maps, list(range(NCORES)), trace=trace)
    return _unshard(res), res


def kernel(**inputs):
    out, _ = _run(inputs, trace=False)
    return out
